# revision 31
# baseline (speedup 1.0000x reference)
"""Causal MQA self-attention (RoPE + RMS-norm on q/k) on 8 TRN2 NeuronCores.

Sharding: core c -> (batch b = c//4, head-group g = c%4 of 4 heads).
Each core computes, for its batch and its 4 heads:
  q/k/v projections -> RoPE -> RMS-norm -> causal attention -> partial
  output projection out_part = attn_out_g @ wo[:, g].T  (shape [S, HID]).
Host sums the 4 per-group partials of each batch (row-parallel matmul
unshard) and stacks the 2 batches.

v2 design notes:
- All transposes run on the DMA xbar (dma_start_transpose), batched 5
  tiles at a time ([s,640] -> [d, 5, 128]), not on the PE.
- RoPE + RMS-norm run in bf16 on the DVE in 2x mode; sin is host-prepped
  as [sin, -sin] so rotate-half is two strided multiplies (no copies).
- Scores diagonal blocks use narrowed rhs (ragged causal trimming); exp
  on fully-causal pairs is fused into [128,1024] instructions.
- Softmax denominator comes from a 129th ones-column on the PV matmul;
  normalization is recip + tensor_scalar (also the bf16 cast).
- Output projection partials are written bf16; host accumulates in f32.
"""

import ml_dtypes
import numpy as np

import concourse.bass as bass
import concourse.mybir as mybir
import concourse.tile as tile
from concourse import bacc
from concourse.bass_utils import run_bass_kernel_spmd

# problem dims (hardcoded per contract)
B, S, HID, H, D = 2, 2048, 2048, 16, 128
NCORES = 8
GROUPS = 4              # head-groups = cores per batch
HG = H // GROUPS        # heads per core
DG = HG * D             # 512 projected q dims per core
NT = S // 128           # 16 sequence tiles
HT = HID // 128         # 16 hidden tiles
NQC = 4                 # q chunks of 512 columns
EPS = 1.1920928955078125e-07
ISD = 1.0 / float(np.sqrt(D))

f32 = mybir.dt.float32
bf16 = mybir.dt.bfloat16

TRACE = False           # test harness may flip this for NTFF profiling
LAST = {}               # last BassKernelResults, for the test harness
PH1_TILES = NT          # bisect knob
PH2_CHUNKS = NQC        # bisect knob

_compiled = None


def _bcast_h(src, nh):
    """AP view of a [128, N] tile as [128, nh, N] with stride-0 head dim."""
    return bass.AP(tensor=src.tensor, offset=src.offset,
                   ap=[src.ap[0], [0, nh], src.ap[-1]])


def _emit(nc, xT, wqT, wkvT, woT, csx, snx, tri, out):
    add = mybir.AluOpType.add
    Sqrt = mybir.ActivationFunctionType.Sqrt
    Exp = mybir.ActivationFunctionType.Exp

    with tile.TileContext(nc) as tc:
        with (
            tc.tile_pool(name="consts", bufs=1) as consts,
            tc.tile_pool(name="bigp", bufs=1) as bigp,
            tc.tile_pool(name="xsp", bufs=10) as xsp,
            tc.tile_pool(name="rsp", bufs=3) as rsp,
            tc.tile_pool(name="smp", bufs=4) as smp,
            tc.tile_pool(name="qnp", bufs=3) as qnp,
            tc.tile_pool(name="ptp", bufs=24) as ptp,
            tc.tile_pool(name="onp", bufs=16) as onp,
            tc.tile_pool(name="otp", bufs=4) as otp,
            tc.tile_pool(name="ocp", bufs=6) as ocp,
            tc.tile_pool(name="pA", bufs=3, space="PSUM") as pA,
            tc.tile_pool(name="pX", bufs=2, space="PSUM") as pX,
        ):
            # ---- constants ----
            eps_t = consts.tile([128, 1], f32)
            nc.vector.memset(eps_t, EPS)
            tri_sb = consts.tile([128, 128], bf16)

            # ---- resident weights / activations ----
            # spread initial loads across DMA queues, first-needed first:
            # wq halves on scalar, wkv/cs/sn on vector, wo (needed last) late
            wqr = wqT.rearrange("(t p) d -> p t d", p=128)
            wkv_sb = bigp.tile([128, HT, 2 * D], bf16, tag="wkv")
            wq_quarters = []
            for qq in range(4):
                wq_q = bigp.tile([128, HT // 4, DG], bf16, tag=f"wq{qq}")
                nc.scalar.dma_start(wq_q, wqr[:, qq * 4:(qq + 1) * 4, :])
                wq_quarters.append(wq_q)
                if qq == 0:
                    nc.scalar.dma_start(
                        wkv_sb, wkvT.rearrange("(t p) d -> p t d", p=128))
            cs_sb = bigp.tile([128, NT, 128], bf16, tag="cs")
            sn_sb = bigp.tile([128, NT, 128], bf16, tag="sn")
            wo_sb = bigp.tile([128, HG, HID], bf16, tag="wo")

            # qkT[d, j, s]: j<4 = transposed-normed q heads, j=4 = k
            qkT = bigp.tile([128, 5, S], bf16, tag="qkT")
            vvb = bigp.tile([128, NT, 132], bf16, tag="vv")   # [s%128, kt, d|ones]
            nc.vector.memset(vvb[:, :, 128:132], 1.0)

            xTr = xT.rearrange("(t p) s -> p t s", p=128)

            xs_tiles = {}
            qkn_tiles = {}

            def emit_qkT_transpose(st):
                qkn = qkn_tiles.pop(st)
                nc.sync.dma_start_transpose(
                    qkT[:, :, st * 128:(st + 1) * 128], qkn)

            def load_x(st):
                sl = slice(st * 128, (st + 1) * 128)
                xs0 = xsp.tile([128, HT // 2, 128], bf16, tag="xs")
                nc.sync.dma_start(xs0, xTr[:, 0:HT // 2, sl])
                xs1 = xsp.tile([128, HT // 2, 128], bf16, tag="xs")
                nc.sync.dma_start(xs1, xTr[:, HT // 2:HT, sl])
                xs_tiles[st] = (xs0, xs1)

            def emit_st(st):
                sl = slice(st * 128, (st + 1) * 128)
                # the deferred transpose's input is ready by now, so it will
                # not block the x loads queued behind it
                if st >= 2:
                    emit_qkT_transpose(st - 2)
                if st + 5 < PH1_TILES:
                    load_x(st + 5)
                xhalves = xs_tiles.pop(st)

                qkv = pA.tile([128, 2, DG], f32, tag="A")
                for t in range(HT):
                    nc.tensor.matmul(
                        qkv[:, 0, :], lhsT=xhalves[t // 8][:, t % 8, :],
                        rhs=wq_quarters[t // 4][:, t % 4, :],
                        start=(t == 0), stop=(t == HT - 1))
                for t in range(HT):
                    nc.tensor.matmul(
                        qkv[:, 1, 0:2 * D], lhsT=xhalves[t // 8][:, t % 8, :],
                        rhs=wkv_sb[:, t, :], start=(t == 0), stop=(t == HT - 1))

                # stage to bf16 SBUF: [0:512]=q heads, [512:640]=k, [640:768]=v
                # kv copy overlaps the q matmuls (disjoint psum regions)
                qkv8 = rsp.tile([128, 768], bf16, tag="qkv8")
                nc.scalar.copy(qkv8[:, DG:DG + 2 * D], qkv[:, 1, 0:2 * D])
                nc.vector.tensor_copy(vvb[:, st, 0:128], qkv8[:, 640:768])
                nc.scalar.copy(qkv8[:, 0:DG], qkv[:, 0, :])

                # ---- RoPE (bf16, DVE 2x) over q heads + k as 5 groups ----
                qk5 = qkv8[:, 0:640].rearrange("p (h t d) -> p h t d", h=5, t=2)
                rot = rsp.tile([128, 640], bf16, tag="rot")
                r5 = rot.rearrange("p (h t d) -> p h t d", h=5, t=2)
                sn_t = sn_sb[:, st, :]
                nc.vector.tensor_mul(r5[:, :, 0, :], qk5[:, :, 1, :],
                                     _bcast_h(sn_t[0:128, 0:64], 5))
                nc.vector.tensor_mul(r5[:, :, 1, :], qk5[:, :, 0, :],
                                     _bcast_h(sn_t[0:128, 64:128], 5))
                t1 = rsp.tile([128, 640], bf16, tag="t1")
                t5 = t1.rearrange("p (h d) -> p h d", h=5)
                nc.vector.tensor_mul(t5, qkv8[:, 0:640].rearrange(
                    "p (h d) -> p h d", h=5), _bcast_h(cs_sb[:, st, :], 5))
                nc.vector.tensor_add(t1, t1, rot)          # t1 = roped qk

                # ---- RMS-norm factors for the 5 groups ----
                nc.vector.tensor_mul(rot, t1, t1)          # rot dead; reuse as sq
                ms5 = smp.tile([128, 5], bf16, tag="ms5")
                with nc.allow_low_precision(reason="rms stats tolerate bf16"):
                    nc.vector.tensor_reduce(
                        ms5, rot.rearrange("p (h d) -> p h d", h=5),
                        axis=mybir.AxisListType.X, op=add)
                srt = smp.tile([128, 5], f32, tag="srt")
                nc.scalar.activation(out=srt, in_=ms5, func=Sqrt,
                                     bias=eps_t[:, 0:1], scale=1.0 / D)
                srtb = smp.tile([128, 5], bf16, tag="srtb")
                with nc.allow_low_precision(reason="rms scale tolerates bf16"):
                    nc.vector.reciprocal(srtb, srt)
                qkn = qnp.tile([128, 640], bf16, tag="qkn")
                nc.vector.tensor_mul(
                    qkn.rearrange("p (h d) -> p h d", h=5),
                    t1.rearrange("p (h d) -> p h d", h=5),
                    bass.AP(tensor=srtb.tensor, offset=srtb.offset,
                            ap=[srtb.ap[0], [1, 5], [0, 128]]))
                qkn_tiles[st] = qkn

            def emit_oproj(qc, stl, otile):
                srow = (4 * qc + stl) * 128
                for cch in range(2):
                    wop = pA.tile([128, 2, DG], f32, tag="A")
                    for cc2 in range(2):
                        for h2 in range(HG):
                            nc.tensor.matmul(
                                wop[:, cc2, :],
                                lhsT=otile[:, h2, stl * 128:(stl + 1) * 128],
                                rhs=wo_sb[:, h2, (2 * cch + cc2) * DG:
                                          (2 * cch + cc2 + 1) * DG],
                                start=(h2 == 0), stop=(h2 == HG - 1))
                    oc = ocp.tile([128, 2 * DG], bf16, tag="oc")
                    if cch == 0:
                        nc.vector.tensor_copy(
                            oc.rearrange("p (a b) -> p a b", a=2), wop)
                    else:
                        nc.scalar.copy(
                            oc.rearrange("p (a b) -> p a b", a=2), wop)
                    nc.gpsimd.dma_start(
                        out[srow:srow + 128,
                            cch * 2 * DG:(cch + 1) * 2 * DG], oc)

            def emit_qc_h(qc, h, on_tiles, otile):
                nkt = 4 * (qc + 1)
                pts = []
                for j2 in range(nkt // 2):
                    sp = pA.tile([128, 2, DG], f32, tag="A")
                    for j in range(2):
                        kt = 2 * j2 + j
                        qoff = max(0, (kt - 4 * qc)) * 128
                        nc.tensor.matmul(
                            sp[:, j, qoff:DG],
                            lhsT=qkT[:, 4, kt * 128:(kt + 1) * 128],
                            rhs=qkT[:, h, qc * DG + qoff:(qc + 1) * DG],
                            start=True, stop=True)
                    pt = ptp.tile([128, 2, DG], bf16, tag="pt")
                    if 2 * j2 + 1 < 4 * qc:     # fully below diagonal
                        nc.scalar.activation(
                            out=pt.rearrange("p a b -> p (a b)"),
                            in_=sp.rearrange("p a b -> p (a b)"),
                            func=Exp, scale=ISD)
                    else:
                        for j in range(2):
                            kt = 2 * j2 + j
                            qoff = max(0, (kt - 4 * qc)) * 128
                            nc.scalar.activation(
                                out=pt[:, j, qoff:DG], in_=sp[:, j, qoff:DG],
                                func=Exp, scale=ISD)
                    pts.append(pt)
                # causal masking of the 4 diagonal k tiles
                for qtl in range(4):
                    kt = 4 * qc + qtl
                    blk = pts[kt // 2][:, kt % 2, qtl * 128:(qtl + 1) * 128]
                    nc.gpsimd.tensor_mul(blk, blk, tri_sb)
                # probs @ [v | ones] per q tile -> [q, d | denom]
                for qtl in range(4):
                    qt = 4 * qc + qtl
                    op = pX.tile([128, DG], f32, tag="X")
                    for kt in range(qt + 1):
                        nc.tensor.matmul(
                            op[:, 0:129],
                            lhsT=pts[kt // 2][:, kt % 2, qtl * 128:(qtl + 1) * 128],
                            rhs=vvb[:, kt, 0:129],
                            start=(kt == 0), stop=(kt == qt))
                    rc = smp.tile([128, 1], f32, tag="rc")
                    nc.vector.reciprocal(rc, op[:, 128:129])
                    nc.vector.tensor_scalar_mul(
                        on_tiles[qtl][:, h, :], op[:, 0:128], rc)
                    if h == HG - 1:
                        # last head: this q tile is complete -> transpose it
                        # to [d, h, q] and run its output-projection row now
                        nc.sync.dma_start_transpose(
                            otile[:, :, qtl * 128:(qtl + 1) * 128],
                            on_tiles[qtl])
                        emit_oproj(qc, qtl, otile)

            def make_chunk_tiles():
                on_tiles = []
                for _qtl in range(4):
                    on_t = onp.tile([128, HG, 128], bf16, tag="on")
                    on_tiles.append(on_t)
                otile = otp.tile([128, HG, DG], bf16, tag="ot")
                return on_tiles, otile

            load_x(0)
            nc.gpsimd.dma_start(cs_sb, csx.rearrange("(t p) d -> p t d", p=128))
            nc.gpsimd.dma_start(sn_sb, snx.rearrange("(t p) d -> p t d", p=128))
            for st in range(1, min(5, PH1_TILES)):
                load_x(st)
            nc.gpsimd.dma_start(tri_sb, tri)
            for st in range(PH1_TILES):
                emit_st(st)
                if st == 2:
                    nc.scalar.dma_start(
                        wo_sb, woT.rearrange("(h p) n -> p h n", p=128))
            # paired chunks: the smaller chunk's matmuls fill the exp-latency
            # bubbles of its partner. qkT deps are tile-granular, so the two
            # trailing transposes are emitted only after the first pair
            # (which needs tiles <= 13) to avoid stalling its scores.
            for qa, qb in ((2, 1), (3, 0)):
                if qb >= PH2_CHUNKS:
                    continue
                ta = make_chunk_tiles()
                tb = make_chunk_tiles()
                for h in range(HG):
                    emit_qc_h(qa, h, *ta)
                    emit_qc_h(qb, h, *tb)
                    if qa == 2 and h == 0:
                        for st in range(max(0, PH1_TILES - 2), PH1_TILES):
                            emit_qkT_transpose(st)


def _build():
    nc = bacc.Bacc("TRN2", target_bir_lowering=False, debug=False,
                   num_devices=NCORES)
    xT = nc.dram_tensor("xT", [HID, S], bf16, kind="ExternalInput").ap()
    wqT = nc.dram_tensor("wqT", [HID, DG], bf16, kind="ExternalInput").ap()
    wkvT = nc.dram_tensor("wkvT", [HID, 2 * D], bf16, kind="ExternalInput").ap()
    woT = nc.dram_tensor("woT", [DG, HID], bf16, kind="ExternalInput").ap()
    csx = nc.dram_tensor("csx", [S, 128], bf16, kind="ExternalInput").ap()
    snx = nc.dram_tensor("snx", [S, 128], bf16, kind="ExternalInput").ap()
    tri = nc.dram_tensor("tri", [128, 128], bf16, kind="ExternalInput").ap()
    out = nc.dram_tensor("out", [S, HID], bf16, kind="ExternalOutput").ap()
    _emit(nc, xT, wqT, wkvT, woT, csx, snx, tri, out)
    nc.compile()
    return nc


def _get_compiled():
    global _compiled
    if _compiled is None:
        _compiled = _build()
    return _compiled


def kernel(x, cos, sin, wq, wk, wv, wo):
    nc = _get_compiled()
    x = np.asarray(x, np.float32)
    cos = np.asarray(cos, np.float32)
    sin = np.asarray(sin, np.float32)
    wq = np.asarray(wq, np.float32)
    wk = np.asarray(wk, np.float32)
    wv = np.asarray(wv, np.float32)
    wo = np.asarray(wo, np.float32)

    bf = ml_dtypes.bfloat16
    wkvT = np.ascontiguousarray(np.concatenate([wk, wv], 0).T.astype(bf))
    csx = np.ascontiguousarray(np.concatenate([cos, cos], 1).astype(bf))
    snx = np.ascontiguousarray(np.concatenate([sin, -sin], 1).astype(bf))
    tri = np.ascontiguousarray(np.triu(np.ones((128, 128), np.float32)).astype(bf))
    xTs = [np.ascontiguousarray(x[b].T.astype(bf)) for b in range(B)]
    wqTs = [np.ascontiguousarray(wq[g * DG:(g + 1) * DG].T.astype(bf))
            for g in range(GROUPS)]
    woTs = [np.ascontiguousarray(wo[:, g * DG:(g + 1) * DG].T.astype(bf))
            for g in range(GROUPS)]

    in_maps = []
    for c in range(NCORES):
        b, g = divmod(c, GROUPS)
        in_maps.append({
            "xT": xTs[b], "wqT": wqTs[g], "wkvT": wkvT, "woT": woTs[g],
            "csx": csx, "snx": snx, "tri": tri,
        })
    res = run_bass_kernel_spmd(nc, in_maps, list(range(NCORES)), trace=TRACE)
    LAST["res"] = res
    outs = [r["out"].astype(np.float32) for r in res.results]
    final = np.empty((B, S, HID), np.float32)
    for b in range(B):
        final[b] = (outs[GROUPS * b] + outs[GROUPS * b + 1]
                    + outs[GROUPS * b + 2] + outs[GROUPS * b + 3])
    return final


# revision 33
# speedup vs baseline: 1.0423x; 1.0423x over previous
"""Causal MQA self-attention (RoPE + RMS-norm on q/k) on 8 TRN2 NeuronCores.

Sharding: core c -> (batch b = c//4, head-group g = c%4 of 4 heads).
Each core computes, for its batch and its 4 heads:
  q/k/v projections -> RoPE -> RMS-norm -> causal attention -> partial
  output projection out_part = attn_out_g @ wo[:, g].T  (shape [S, HID]).
Host sums the 4 per-group partials of each batch (row-parallel matmul
unshard) and stacks the 2 batches.

v2 design notes:
- All transposes run on the DMA xbar (dma_start_transpose), batched 5
  tiles at a time ([s,640] -> [d, 5, 128]), not on the PE.
- RoPE + RMS-norm run in bf16 on the DVE in 2x mode; sin is host-prepped
  as [sin, -sin] so rotate-half is two strided multiplies (no copies).
- Scores diagonal blocks use narrowed rhs (ragged causal trimming); exp
  on fully-causal pairs is fused into [128,1024] instructions.
- Softmax denominator comes from a 129th ones-column on the PV matmul;
  normalization is recip + tensor_scalar (also the bf16 cast).
- Output projection partials are written bf16; host accumulates in f32.
"""

import ml_dtypes
import numpy as np

import concourse.bass as bass
import concourse.mybir as mybir
import concourse.tile as tile
from concourse import bacc
from concourse.bass_utils import run_bass_kernel_spmd

# problem dims (hardcoded per contract)
B, S, HID, H, D = 2, 2048, 2048, 16, 128
NCORES = 8
GROUPS = 4              # head-groups = cores per batch
HG = H // GROUPS        # heads per core
DG = HG * D             # 512 projected q dims per core
NT = S // 128           # 16 sequence tiles
HT = HID // 128         # 16 hidden tiles
NQC = 4                 # q chunks of 512 columns
EPS = 1.1920928955078125e-07
ISD = 1.0 / float(np.sqrt(D))

f32 = mybir.dt.float32
bf16 = mybir.dt.bfloat16

TRACE = False           # test harness may flip this for NTFF profiling
LAST = {}               # last BassKernelResults, for the test harness
PH1_TILES = NT          # bisect knob
PH2_CHUNKS = NQC        # bisect knob

_compiled = None


def _bcast_h(src, nh):
    """AP view of a [128, N] tile as [128, nh, N] with stride-0 head dim."""
    return bass.AP(tensor=src.tensor, offset=src.offset,
                   ap=[src.ap[0], [0, nh], src.ap[-1]])


def _emit(nc, xT, wqT, wkvT, woT, csx, snx, tri, out):
    add = mybir.AluOpType.add
    Sqrt = mybir.ActivationFunctionType.Sqrt
    Exp = mybir.ActivationFunctionType.Exp

    with tile.TileContext(nc) as tc:
        with (
            tc.tile_pool(name="consts", bufs=1) as consts,
            tc.tile_pool(name="bigp", bufs=1) as bigp,
            tc.tile_pool(name="xsp", bufs=10) as xsp,
            tc.tile_pool(name="rsp", bufs=3) as rsp,
            tc.tile_pool(name="smp", bufs=4) as smp,
            tc.tile_pool(name="qnp", bufs=3) as qnp,
            tc.tile_pool(name="ptp", bufs=24) as ptp,
            tc.tile_pool(name="onp", bufs=16) as onp,
            tc.tile_pool(name="otp", bufs=4) as otp,
            tc.tile_pool(name="ocp", bufs=6) as ocp,
            tc.tile_pool(name="pA", bufs=3, space="PSUM") as pA,
            tc.tile_pool(name="pX", bufs=2, space="PSUM") as pX,
        ):
            # ---- constants ----
            eps_t = consts.tile([128, 1], f32)
            nc.vector.memset(eps_t, EPS)
            tri_sb = consts.tile([128, 128], bf16)

            # ---- resident weights / activations ----
            # spread initial loads across DMA queues, first-needed first:
            # wq halves on scalar, wkv/cs/sn on vector, wo (needed last) late
            wqr = wqT.rearrange("(t p) d -> p t d", p=128)
            wkv_sb = bigp.tile([128, HT, 2 * D], bf16, tag="wkv")
            wq_quarters = []
            for qq in range(4):
                wq_q = bigp.tile([128, HT // 4, DG], bf16, tag=f"wq{qq}")
                nc.scalar.dma_start(wq_q, wqr[:, qq * 4:(qq + 1) * 4, :])
                wq_quarters.append(wq_q)
                if qq == 0:
                    nc.scalar.dma_start(
                        wkv_sb, wkvT.rearrange("(t p) d -> p t d", p=128))
            cs_sb = bigp.tile([128, NT, 128], bf16, tag="cs")
            sn_sb = bigp.tile([128, NT, 128], bf16, tag="sn")
            wo_sb = bigp.tile([128, HG, HID], bf16, tag="wo")

            # qkT[d, j, s]: j<4 = transposed-normed q heads, j=4 = k
            qkT = bigp.tile([128, 5, S], bf16, tag="qkT")
            vvb = bigp.tile([128, NT, 132], bf16, tag="vv")   # [s%128, kt, d|ones]
            nc.vector.memset(vvb[:, :, 128:132], 1.0)

            xTr = xT.rearrange("(t p) s -> p t s", p=128)

            xs_tiles = {}
            qkn_tiles = {}

            def emit_qkT_transpose(st):
                qkn = qkn_tiles.pop(st)
                nc.sync.dma_start_transpose(
                    qkT[:, :, st * 128:(st + 1) * 128], qkn)

            def load_x(st):
                sl = slice(st * 128, (st + 1) * 128)
                xs0 = xsp.tile([128, HT // 2, 128], bf16, tag="xs")
                nc.sync.dma_start(xs0, xTr[:, 0:HT // 2, sl])
                xs1 = xsp.tile([128, HT // 2, 128], bf16, tag="xs")
                nc.sync.dma_start(xs1, xTr[:, HT // 2:HT, sl])
                xs_tiles[st] = (xs0, xs1)

            def emit_st(st):
                sl = slice(st * 128, (st + 1) * 128)
                # the deferred transpose's input is ready by now, so it will
                # not block the x loads queued behind it
                if st >= 2:
                    emit_qkT_transpose(st - 2)
                if st + 5 < PH1_TILES:
                    load_x(st + 5)
                xhalves = xs_tiles.pop(st)

                qkv = pA.tile([128, 2, DG], f32, tag="A")
                for t in range(HT):
                    nc.tensor.matmul(
                        qkv[:, 0, :], lhsT=xhalves[t // 8][:, t % 8, :],
                        rhs=wq_quarters[t // 4][:, t % 4, :],
                        start=(t == 0), stop=(t == HT - 1))
                for t in range(HT):
                    nc.tensor.matmul(
                        qkv[:, 1, 0:2 * D], lhsT=xhalves[t // 8][:, t % 8, :],
                        rhs=wkv_sb[:, t, :], start=(t == 0), stop=(t == HT - 1))

                # stage to bf16 SBUF: [0:512]=q heads, [512:640]=k, [640:768]=v
                # kv copy overlaps the q matmuls (disjoint psum regions)
                qkv8 = rsp.tile([128, 768], bf16, tag="qkv8")
                nc.scalar.copy(qkv8[:, DG:DG + 2 * D], qkv[:, 1, 0:2 * D])
                nc.vector.tensor_copy(vvb[:, st, 0:128], qkv8[:, 640:768])
                nc.scalar.copy(qkv8[:, 0:DG], qkv[:, 0, :])

                # ---- RoPE (bf16, DVE 2x) over q heads + k as 5 groups ----
                qk5 = qkv8[:, 0:640].rearrange("p (h t d) -> p h t d", h=5, t=2)
                rot = rsp.tile([128, 640], bf16, tag="rot")
                r5 = rot.rearrange("p (h t d) -> p h t d", h=5, t=2)
                sn_t = sn_sb[:, st, :]
                nc.vector.tensor_mul(r5[:, :, 0, :], qk5[:, :, 1, :],
                                     _bcast_h(sn_t[0:128, 0:64], 5))
                nc.vector.tensor_mul(r5[:, :, 1, :], qk5[:, :, 0, :],
                                     _bcast_h(sn_t[0:128, 64:128], 5))
                t1 = rsp.tile([128, 640], bf16, tag="t1")
                t5 = t1.rearrange("p (h d) -> p h d", h=5)
                nc.vector.tensor_mul(t5, qkv8[:, 0:640].rearrange(
                    "p (h d) -> p h d", h=5), _bcast_h(cs_sb[:, st, :], 5))
                nc.vector.tensor_add(t1, t1, rot)          # t1 = roped qk

                # ---- RMS-norm factors for the 5 groups ----
                nc.vector.tensor_mul(rot, t1, t1)          # rot dead; reuse as sq
                ms5 = smp.tile([128, 5], bf16, tag="ms5")
                with nc.allow_low_precision(reason="rms stats tolerate bf16"):
                    nc.vector.tensor_reduce(
                        ms5, rot.rearrange("p (h d) -> p h d", h=5),
                        axis=mybir.AxisListType.X, op=add)
                srt = smp.tile([128, 5], f32, tag="srt")
                nc.scalar.activation(out=srt, in_=ms5, func=Sqrt,
                                     bias=eps_t[:, 0:1], scale=1.0 / D)
                srtb = smp.tile([128, 5], bf16, tag="srtb")
                with nc.allow_low_precision(reason="rms scale tolerates bf16"):
                    nc.vector.reciprocal(srtb, srt)
                qkn = qnp.tile([128, 640], bf16, tag="qkn")
                nc.vector.tensor_mul(
                    qkn.rearrange("p (h d) -> p h d", h=5),
                    t1.rearrange("p (h d) -> p h d", h=5),
                    bass.AP(tensor=srtb.tensor, offset=srtb.offset,
                            ap=[srtb.ap[0], [1, 5], [0, 128]]))
                qkn_tiles[st] = qkn

            def emit_oproj(qc, stl, otile):
                srow = (4 * qc + stl) * 128
                for cch in range(2):
                    wop = pA.tile([128, 2, DG], f32, tag="A")
                    for cc2 in range(2):
                        for h2 in range(HG):
                            nc.tensor.matmul(
                                wop[:, cc2, :],
                                lhsT=otile[:, h2, stl * 128:(stl + 1) * 128],
                                rhs=wo_sb[:, h2, (2 * cch + cc2) * DG:
                                          (2 * cch + cc2 + 1) * DG],
                                start=(h2 == 0), stop=(h2 == HG - 1))
                    oc = ocp.tile([128, 2 * DG], bf16, tag="oc")
                    if cch == 0:
                        nc.vector.tensor_copy(
                            oc.rearrange("p (a b) -> p a b", a=2), wop)
                    else:
                        nc.scalar.copy(
                            oc.rearrange("p (a b) -> p a b", a=2), wop)
                    nc.gpsimd.dma_start(
                        out[srow:srow + 128,
                            cch * 2 * DG:(cch + 1) * 2 * DG], oc)

            def emit_qc_h(qc, h, on_tiles, otile):
                nkt = 4 * (qc + 1)
                pts = []
                for j2 in range(nkt // 2):
                    sp = pA.tile([128, 2, DG], f32, tag="A")
                    for j in range(2):
                        kt = 2 * j2 + j
                        qoff = max(0, (kt - 4 * qc)) * 128
                        nc.tensor.matmul(
                            sp[:, j, qoff:DG],
                            lhsT=qkT[:, 4, kt * 128:(kt + 1) * 128],
                            rhs=qkT[:, h, qc * DG + qoff:(qc + 1) * DG],
                            start=True, stop=True)
                    pt = ptp.tile([128, 2, DG], bf16, tag="pt")
                    if 2 * j2 + 1 < 4 * qc:     # fully below diagonal
                        nc.scalar.activation(
                            out=pt.rearrange("p a b -> p (a b)"),
                            in_=sp.rearrange("p a b -> p (a b)"),
                            func=Exp, scale=ISD)
                    else:
                        for j in range(2):
                            kt = 2 * j2 + j
                            qoff = max(0, (kt - 4 * qc)) * 128
                            nc.scalar.activation(
                                out=pt[:, j, qoff:DG], in_=sp[:, j, qoff:DG],
                                func=Exp, scale=ISD)
                    pts.append(pt)
                # causal masking of the 4 diagonal k tiles
                for qtl in range(4):
                    kt = 4 * qc + qtl
                    blk = pts[kt // 2][:, kt % 2, qtl * 128:(qtl + 1) * 128]
                    nc.vector.tensor_mul(blk, blk, tri_sb)
                # probs @ [v | ones] per q tile -> [q, d | denom]
                for qtl in range(4):
                    qt = 4 * qc + qtl
                    op = pX.tile([128, DG], f32, tag="X")
                    for kt in range(qt + 1):
                        nc.tensor.matmul(
                            op[:, 0:129],
                            lhsT=pts[kt // 2][:, kt % 2, qtl * 128:(qtl + 1) * 128],
                            rhs=vvb[:, kt, 0:129],
                            start=(kt == 0), stop=(kt == qt))
                    rc = smp.tile([128, 1], f32, tag="rc")
                    nc.vector.reciprocal(rc, op[:, 128:129])
                    nc.vector.tensor_scalar_mul(
                        on_tiles[qtl][:, h, :], op[:, 0:128], rc)
                    if h == HG - 1:
                        # last head: this q tile is complete -> transpose it
                        # to [d, h, q] and run its output-projection row now
                        nc.sync.dma_start_transpose(
                            otile[:, :, qtl * 128:(qtl + 1) * 128],
                            on_tiles[qtl])
                        emit_oproj(qc, qtl, otile)

            def make_chunk_tiles():
                on_tiles = []
                for _qtl in range(4):
                    on_t = onp.tile([128, HG, 128], bf16, tag="on")
                    on_tiles.append(on_t)
                otile = otp.tile([128, HG, DG], bf16, tag="ot")
                return on_tiles, otile

            load_x(0)
            nc.gpsimd.dma_start(cs_sb, csx.rearrange("(t p) d -> p t d", p=128))
            nc.gpsimd.dma_start(sn_sb, snx.rearrange("(t p) d -> p t d", p=128))
            for st in range(1, min(5, PH1_TILES)):
                load_x(st)
            nc.gpsimd.dma_start(tri_sb, tri)
            for st in range(PH1_TILES):
                emit_st(st)
                if st == 2:
                    nc.scalar.dma_start(
                        wo_sb, woT.rearrange("(h p) n -> p h n", p=128))
            # paired chunks: the smaller chunk's matmuls fill the exp-latency
            # bubbles of its partner. qkT deps are tile-granular, so the two
            # trailing transposes are emitted only after the first pair
            # (which needs tiles <= 13) to avoid stalling its scores.
            for qa, qb in ((1, 2), (0, 3)):
                if qb >= PH2_CHUNKS:
                    continue
                ta = make_chunk_tiles()
                tb = make_chunk_tiles()
                for h in range(HG):
                    emit_qc_h(qa, h, *ta)
                    emit_qc_h(qb, h, *tb)
                    if qa == 1 and h == 0:
                        for st in range(max(0, PH1_TILES - 2), PH1_TILES):
                            emit_qkT_transpose(st)


def _build():
    nc = bacc.Bacc("TRN2", target_bir_lowering=False, debug=False,
                   num_devices=NCORES)
    xT = nc.dram_tensor("xT", [HID, S], bf16, kind="ExternalInput").ap()
    wqT = nc.dram_tensor("wqT", [HID, DG], bf16, kind="ExternalInput").ap()
    wkvT = nc.dram_tensor("wkvT", [HID, 2 * D], bf16, kind="ExternalInput").ap()
    woT = nc.dram_tensor("woT", [DG, HID], bf16, kind="ExternalInput").ap()
    csx = nc.dram_tensor("csx", [S, 128], bf16, kind="ExternalInput").ap()
    snx = nc.dram_tensor("snx", [S, 128], bf16, kind="ExternalInput").ap()
    tri = nc.dram_tensor("tri", [128, 128], bf16, kind="ExternalInput").ap()
    out = nc.dram_tensor("out", [S, HID], bf16, kind="ExternalOutput").ap()
    _emit(nc, xT, wqT, wkvT, woT, csx, snx, tri, out)
    nc.compile()
    return nc


def _get_compiled():
    global _compiled
    if _compiled is None:
        _compiled = _build()
    return _compiled


def kernel(x, cos, sin, wq, wk, wv, wo):
    nc = _get_compiled()
    x = np.asarray(x, np.float32)
    cos = np.asarray(cos, np.float32)
    sin = np.asarray(sin, np.float32)
    wq = np.asarray(wq, np.float32)
    wk = np.asarray(wk, np.float32)
    wv = np.asarray(wv, np.float32)
    wo = np.asarray(wo, np.float32)

    bf = ml_dtypes.bfloat16
    wkvT = np.ascontiguousarray(np.concatenate([wk, wv], 0).T.astype(bf))
    csx = np.ascontiguousarray(np.concatenate([cos, cos], 1).astype(bf))
    snx = np.ascontiguousarray(np.concatenate([sin, -sin], 1).astype(bf))
    tri = np.ascontiguousarray(np.triu(np.ones((128, 128), np.float32)).astype(bf))
    xTs = [np.ascontiguousarray(x[b].T.astype(bf)) for b in range(B)]
    wqTs = [np.ascontiguousarray(wq[g * DG:(g + 1) * DG].T.astype(bf))
            for g in range(GROUPS)]
    woTs = [np.ascontiguousarray(wo[:, g * DG:(g + 1) * DG].T.astype(bf))
            for g in range(GROUPS)]

    in_maps = []
    for c in range(NCORES):
        b, g = divmod(c, GROUPS)
        in_maps.append({
            "xT": xTs[b], "wqT": wqTs[g], "wkvT": wkvT, "woT": woTs[g],
            "csx": csx, "snx": snx, "tri": tri,
        })
    res = run_bass_kernel_spmd(nc, in_maps, list(range(NCORES)), trace=TRACE)
    LAST["res"] = res
    outs = [r["out"].astype(np.float32) for r in res.results]
    final = np.empty((B, S, HID), np.float32)
    for b in range(B):
        final[b] = (outs[GROUPS * b] + outs[GROUPS * b + 1]
                    + outs[GROUPS * b + 2] + outs[GROUPS * b + 3])
    return final
